# revision 3
# baseline (speedup 1.0000x reference)
"""GQA attention (B=2, N=2048, D=2048, H=16, KVH=4) on 8 trn2 cores.

Sharding: core c -> (batch b = c//4, kv-group g = c%4). Each core computes
its 4 q-heads / 1 kv-head slice end-to-end (qkv proj + rope + causal
attention + o_proj partial); the host sums the 4 partials per batch.

All matmuls run as float32r (full PE rate, ~1e-4 rel precision).
Attention is computed in S^T layout ([tok_j, tok_i]) so that:
  - PV contraction uses V in natural layout as lhsT (no P transpose),
  - softmax denominators come from a ones-column matmul,
  - normalization happens on Ct via a K=1 broadcast matmul of 1/denom.
Causal masking adds -1e9 tiles into PSUM via an identity-matmul before
the scores accumulate; exp() then zeroes them (no max-subtraction needed:
scores are O(+-6) for this distribution).
"""

import sys

sys.path.insert(0, "/opt/trn_rl_repo")

import numpy as np
from contextlib import ExitStack

B, N, D = 2, 2048, 2048
H, KVH = 16, 4
DH = 128
HPC = 4          # q heads per core
GQ = 512         # q cols per core
ROPE_BASE = 10000.0
NEG = -1.0e9
SCALE = 1.0 / np.sqrt(DH)

_CACHE = {}


def _build_nc():
    import concourse.tile as tile
    from concourse import bacc, mybir

    f32 = mybir.dt.float32
    f32r = mybir.dt.float32r
    EXP = mybir.ActivationFunctionType.Exp

    nc = bacc.Bacc("TRN2", target_bir_lowering=False, debug=False)

    xt = nc.dram_tensor("xt", [D, N], f32, kind="ExternalInput").ap()
    wqkv = nc.dram_tensor("wqkv", [D, GQ + 2 * DH], f32, kind="ExternalInput").ap()
    wo = nc.dram_tensor("wo", [GQ, D], f32, kind="ExternalInput").ap()
    cost = nc.dram_tensor("cost", [DH, N], f32, kind="ExternalInput").ap()
    sint = nc.dram_tensor("sint", [DH, N], f32, kind="ExternalInput").ap()
    rt = nc.dram_tensor("rt", [DH, DH], f32, kind="ExternalInput").ap()
    masks = nc.dram_tensor("masks", [128, 4 * 512], f32, kind="ExternalInput").ap()
    ident = nc.dram_tensor("ident", [128, 128], f32, kind="ExternalInput").ap()
    ones = nc.dram_tensor("ones", [128, 128], f32, kind="ExternalInput").ap()
    out = nc.dram_tensor("out", [N, D], f32, kind="ExternalOutput").ap()

    xt_r = xt.rearrange("(kd p) t -> p kd t", p=128)      # [128, 16, 2048]
    wqkv_r = wqkv.rearrange("(kd p) c -> p kd c", p=128)  # [128, 16, 768]
    wo_r = wo.rearrange("(h p) n -> p h n", p=128)        # [128, 4, 2048]
    out_r = out.rearrange("(it p) n -> p it n", p=128)    # [128, 16, 2048]
    masks_r = masks.rearrange("p (v i) -> p v i", v=4)    # [128, 4, 512]

    with tile.TileContext(nc) as tc, ExitStack() as ctx:
        sing = ctx.enter_context(tc.tile_pool(name="sing", bufs=1))
        xtp = ctx.enter_context(tc.tile_pool(name="xtp", bufs=2))
        wqp = ctx.enter_context(tc.tile_pool(name="wqp", bufs=2))
        wop = ctx.enter_context(tc.tile_pool(name="wop", bufs=2))
        rawp = ctx.enter_context(tc.tile_pool(name="rawp", bufs=2))
        ropep = ctx.enter_context(tc.tile_pool(name="ropep", bufs=4))
        etp = ctx.enter_context(tc.tile_pool(name="etp", bufs=3))
        rbp = ctx.enter_context(tc.tile_pool(name="rbp", bufs=2))
        recp = ctx.enter_context(tc.tile_pool(name="recp", bufs=2))
        outp = ctx.enter_context(tc.tile_pool(name="outp", bufs=3))
        psp = ctx.enter_context(tc.tile_pool(name="psp", bufs=8, space="PSUM"))

        def ps_tile():
            return psp.tile([128, 512], f32, tag="ps", name="ps")

        # persistent SBUF tensors
        qt = sing.tile([128, HPC, N], f32)    # roped Q^T per head  [dh, tok]
        kt = sing.tile([128, N], f32)         # roped K^T           [dh, tok]
        vn = sing.tile([128, N], f32)         # V natural tiles     [tok-in-tile, dh]
        ct = sing.tile([128, HPC, N], f32)    # normalized ctx^T    [dh, tok]
        cost_sb = sing.tile([DH, N], f32)
        sint_sb = sing.tile([DH, N], f32)
        rt_sb = sing.tile([DH, DH], f32)
        masks_sb = sing.tile([128, 4, 512], f32)
        id_sb = sing.tile([128, 128], f32)
        ones_sb = sing.tile([128, 128], f32)

        nc.sync.dma_start(out=cost_sb, in_=cost)
        nc.sync.dma_start(out=sint_sb, in_=sint)
        nc.sync.dma_start(out=rt_sb[:].bitcast(f32r), in_=rt.bitcast(f32r))
        nc.sync.dma_start(out=masks_sb[:].bitcast(f32r), in_=masks_r.bitcast(f32r))
        nc.sync.dma_start(out=id_sb[:].bitcast(f32r), in_=ident.bitcast(f32r))
        nc.sync.dma_start(out=ones_sb[:].bitcast(f32r), in_=ones.bitcast(f32r))

        # ---------------- Phase A: projections + rope -------------------
        for tc4 in range(4):
            tsl = slice(tc4 * 512, (tc4 + 1) * 512)
            proj = [ps_tile() for _ in range(6)]
            for kq in range(4):
                xt_t = xtp.tile([128, 4, 512], f32)
                nc.sync.dma_start(
                    out=xt_t[:].bitcast(f32r),
                    in_=xt_r[:, kq * 4 : kq * 4 + 4, tsl].bitcast(f32r),
                )
                wq_t = wqp.tile([128, 4, 768], f32)
                nc.sync.dma_start(
                    out=wq_t[:].bitcast(f32r),
                    in_=wqkv_r[:, kq * 4 : kq * 4 + 4, :].bitcast(f32r),
                )
                for m in range(6):
                    for k4 in range(4):
                        kd = kq * 4 + k4
                        nc.tensor.matmul(
                            proj[m],
                            lhsT=wq_t[:, k4, m * 128 : (m + 1) * 128].bitcast(f32r),
                            rhs=xt_t[:, k4, :].bitcast(f32r),
                            start=(kd == 0),
                            stop=(kd == 15),
                        )
            for m in range(6):
                raw = rawp.tile([128, 512], f32)
                nc.scalar.copy(raw[:].bitcast(f32r), proj[m])
                if m < 5:  # q heads + k: rope
                    rot = ps_tile()
                    nc.tensor.matmul(
                        rot,
                        lhsT=rt_sb[:].bitcast(f32r),
                        rhs=raw[:].bitcast(f32r),
                        start=True,
                        stop=True,
                    )
                    t1 = ropep.tile([128, 512], f32, tag="rope_t")
                    nc.vector.tensor_mul(t1, raw, cost_sb[:, tsl])
                    t2 = ropep.tile([128, 512], f32, tag="rope_t")
                    nc.vector.tensor_mul(t2, rot, sint_sb[:, tsl])
                    dest = qt[:, m, tsl] if m < 4 else kt[:, tsl]
                    nc.vector.tensor_add(dest.bitcast(f32r), t1, t2)
                else:  # v: transpose to natural layout
                    for s in range(4):
                        tp = ps_tile()
                        nc.tensor.transpose(
                            tp[:, 0:128], raw[:, s * 128 : (s + 1) * 128], id_sb[:]
                        )
                        jt = tc4 * 4 + s
                        nc.scalar.copy(
                            vn[:, jt * 128 : (jt + 1) * 128].bitcast(f32r),
                            tp[:, 0:128],
                        )

        # ---------------- Phase B: attention ---------------------------
        for h in range(HPC):
            for ic in range(4):
                isl = slice(ic * 512, (ic + 1) * 512)
                njt = 4 * (ic + 1)
                ct_ps = ps_tile()
                den_ps = ps_tile()
                for jt in range(njt):
                    st = ps_tile()
                    diag = jt >= ic * 4
                    if diag:
                        nc.tensor.matmul(
                            st,
                            lhsT=id_sb[:].bitcast(f32r),
                            rhs=masks_sb[:, jt - ic * 4, :].bitcast(f32r),
                            start=True,
                            stop=False,
                        )
                    nc.tensor.matmul(
                        st,
                        lhsT=kt[:, jt * 128 : (jt + 1) * 128].bitcast(f32r),
                        rhs=qt[:, h, isl].bitcast(f32r),
                        start=not diag,
                        stop=True,
                    )
                    et = etp.tile([128, 512], f32)
                    nc.scalar.activation(et[:].bitcast(f32r), st, EXP, scale=SCALE)
                    nc.tensor.matmul(
                        ct_ps,
                        lhsT=vn[:, jt * 128 : (jt + 1) * 128].bitcast(f32r),
                        rhs=et[:].bitcast(f32r),
                        start=(jt == 0),
                        stop=(jt == njt - 1),
                    )
                    nc.tensor.matmul(
                        den_ps[0:1, :],
                        lhsT=ones_sb[:, 0:1].bitcast(f32r),
                        rhs=et[:].bitcast(f32r),
                        start=(jt == 0),
                        stop=(jt == njt - 1),
                    )
                rec = recp.tile([1, 512], f32)
                with nc.allow_low_precision(reason="f32r bits are f32"):
                    nc.vector.reciprocal(rec[:].bitcast(f32r), den_ps[0:1, :])
                rb_ps = ps_tile()
                nc.tensor.matmul(
                    rb_ps,
                    lhsT=ones_sb[0:1, :].bitcast(f32r),
                    rhs=rec[:].bitcast(f32r),
                    start=True,
                    stop=True,
                )
                rb = rbp.tile([128, 512], f32)
                nc.scalar.copy(rb, rb_ps)
                nc.vector.tensor_mul(ct[:, h, isl].bitcast(f32r), ct_ps, rb)

        # ---------------- Phase C: o_proj ------------------------------
        for ncol in range(4):
            nsl = slice(ncol * 512, (ncol + 1) * 512)
            wo_t = wop.tile([128, 4, 512], f32)
            nc.sync.dma_start(
                out=wo_t[:].bitcast(f32r), in_=wo_r[:, :, nsl].bitcast(f32r)
            )
            for it in range(16):
                op = ps_tile()
                for h in range(HPC):
                    nc.tensor.matmul(
                        op,
                        lhsT=ct[:, h, it * 128 : (it + 1) * 128].bitcast(f32r),
                        rhs=wo_t[:, h, :].bitcast(f32r),
                        start=(h == 0),
                        stop=(h == 3),
                    )
                oc = outp.tile([128, 512], f32)
                nc.vector.tensor_copy(oc, op)
                nc.sync.dma_start(out=out_r[:, it, nsl], in_=oc)

    nc.compile()
    return nc


def _host_inputs(x, wq, wk, wv, wo):
    n = np.arange(N, dtype=np.float64)
    inv_freq = 1.0 / (ROPE_BASE ** (np.arange(0, DH, 2, dtype=np.float64) / DH))
    ang = n[:, None] * inv_freq[None, :]
    ang = np.concatenate([ang, ang], axis=-1)  # [N, DH]
    cost = np.cos(ang).T.astype(np.float32).copy()  # [DH, N]
    sint = np.sin(ang).T.astype(np.float32).copy()

    R = np.zeros((DH, DH), dtype=np.float32)
    half = DH // 2
    R[np.arange(half), np.arange(half) + half] = -1.0
    R[np.arange(half) + half, np.arange(half)] = 1.0
    rt = np.ascontiguousarray(R.T)

    j = np.arange(128)[:, None]
    i = np.arange(512)[None, :]
    masks = np.zeros((128, 4, 512), dtype=np.float32)
    for v in range(4):
        masks[:, v, :] = np.where(j + v * 128 > i, NEG, 0.0)
    masks = masks.reshape(128, 4 * 512)

    ident = np.eye(128, dtype=np.float32)
    ones = np.ones((128, 128), dtype=np.float32)

    in_maps = []
    for c in range(8):
        b, g = c // 4, c % 4
        in_maps.append(
            {
                "xt": np.ascontiguousarray(x[b].T),
                "wqkv": np.ascontiguousarray(
                    np.concatenate(
                        [
                            wq[:, g * GQ : (g + 1) * GQ],
                            wk[:, g * DH : (g + 1) * DH],
                            wv[:, g * DH : (g + 1) * DH],
                        ],
                        axis=1,
                    )
                ),
                "wo": np.ascontiguousarray(wo[g * GQ : (g + 1) * GQ, :]),
                "cost": cost,
                "sint": sint,
                "rt": rt,
                "masks": masks,
                "ident": ident,
                "ones": ones,
            }
        )
    return in_maps


def kernel(x, wq, wk, wv, wo):
    from concourse.bass_utils import run_bass_kernel_spmd

    if "nc" not in _CACHE:
        _CACHE["nc"] = _build_nc()
    nc = _CACHE["nc"]
    in_maps = _host_inputs(
        np.asarray(x), np.asarray(wq), np.asarray(wk), np.asarray(wv), np.asarray(wo)
    )
    res = run_bass_kernel_spmd(nc, in_maps, list(range(8)), trace=False)
    out = np.zeros((B, N, D), dtype=np.float32)
    for c in range(8):
        out[c // 4] += res.results[c]["out"]
    return out
